# revision 39
# baseline (speedup 1.0000x reference)
"""GCN MixturePredictor kernel for 8 Trainium2 NeuronCores.

Strategy (everything heavy on device; host only sorts/indexes):
- Graphs sharded 4096/core. Node ids re-laid into padded "slots": per core
  32 windows (128 graphs each) x WT tiles x 128 slots, so pooling windows are
  tile-aligned and fully static.
- hp = (x @ W_gcn) * rsqrt(deg+1) computed on host (tiny GEMM + fused numba
  bf16 cast), sharded 1/8 per core, AllGathered on device to a 1M-row table.
- Edges sorted by destination slot with a packed int64 np.sort (radix, fast),
  then scattered (numba, sequential ranks) into per-tile padded blocks of
  EMAX x 128. Each edge is packed as (dstloc << 20) | src into int32;
  dummy slots carry dstloc=200 so their one-hot row is all-zero.
- Device per 128-node tile: per 128-edge block one indirect DMA gathers
  hp[src] rows into a [128, 33] tile (col 32 preset to 1.0); one-hot
  matrices (is_equal vs an iota row) feed EMAX accumulating bf16 matmuls
  into PSUM. Column 32 of the accumulator yields the in-degree, from which
  dinv = 1/sqrt(deg+1) is computed on device; combine with the gathered
  self row, add bias, tanh -> hout (bf16).
- Pooling per 128-graph window via the same one-hot matmul over its WT
  tiles (dummy slots have bl=255 and never match); classifier matmul on
  device. Output [32768, 109] (bf16 on the wire, f32 returned).
- kernel() uses a cached jitted executable and device_puts each global array as
  soon as host prep produces it, overlapping transfer with prep; falls back
  to bass_utils.run_bass_kernel_spmd if that path is unavailable.
"""

import numpy as np
import ml_dtypes

BF16 = ml_dtypes.bfloat16

try:
    import numba

    @numba.njit(cache=True, fastmath=False)
    def _nb_pack_deg(src, dst, slot, n_nodes):
        E = src.shape[0]
        packed = np.empty(E, np.int64)
        deg = np.zeros(n_nodes, np.float32)
        for e in range(E):
            d = dst[e]
            deg[d] += 1.0
            packed[e] = (np.int64(slot[d]) << 20) | np.int64(src[e])
        return packed, deg

    @numba.njit(cache=True, fastmath=False)
    def _nb_scale_bf16(h, dinv, out_u16):
        n, c = h.shape
        buf = np.empty(c, np.float32)
        for i in range(n):
            dv = dinv[i]
            for j in range(c):
                buf[j] = h[i, j] * dv
            bu = buf.view(np.uint32)
            for j in range(c):
                b = bu[j]
                out_u16[i, j] = np.uint16(
                    (b + np.uint32(0x7FFF) + ((b >> np.uint32(16)) & np.uint32(1)))
                    >> np.uint32(16))

    @numba.njit(cache=True, fastmath=False)
    def _nb_scale_fp8(h, dinv, out_u8):
        n, c = h.shape
        buf = np.empty(c, np.float32)
        for i in range(n):
            dv = dinv[i]
            for j in range(c):
                buf[j] = h[i, j] * dv
            bu = buf.view(np.uint32)
            for j in range(c):
                b = bu[j]
                s = np.uint8((b >> np.uint32(24)) & np.uint32(0x80))
                e = np.int64((b >> np.uint32(23)) & np.uint32(0xFF))
                m = np.int64(b & np.uint32(0x7FFFFF))
                te = e - 120          # biased target exponent (bias 7)
                if e == 0 or te < -3:
                    out_u8[i, j] = s
                elif te <= 0:
                    full = m | 0x800000
                    shift = 20 + 1 - te
                    half = np.int64(1) << (shift - 1)
                    r = (full + (half - 1) + ((full >> shift) & 1)) >> shift
                    out_u8[i, j] = s | np.uint8(r)
                elif te >= 15:
                    out_u8[i, j] = s | np.uint8(0x77)   # saturate (never hit)
                else:
                    r = (m + 0x7FFFF + ((m >> 20) & 1)) >> 20
                    out_u8[i, j] = s | np.uint8((te << 3) + r)

    @numba.njit(cache=True, fastmath=False)
    def _nb_quant_i8(h, dinv, out_i8, scales):
        n, c = h.shape
        for j in range(c):
            scales[j] = 0.0
        for i in range(n):
            dv = dinv[i]
            for j in range(c):
                v = abs(h[i, j] * dv)
                if v > scales[j]:
                    scales[j] = v
        inv = np.empty(c, np.float32)
        for j in range(c):
            if scales[j] <= 0.0:
                scales[j] = 1.0
            scales[j] = scales[j] / 127.0
            inv[j] = 1.0 / scales[j]
        for i in range(n):
            dv = dinv[i]
            for j in range(c):
                q = np.int32(np.floor(h[i, j] * dv * inv[j] + 0.5))
                if q > 127:
                    q = 127
                elif q < -127:
                    q = -127
                out_i8[i, j] = np.int8(q)

    @numba.njit(cache=True, fastmath=False)
    def _nb_edge_scatter_core(packed, ed, NT, EMAX, EB, tile_base):
        E = packed.shape[0]
        prev_tile = np.int64(-1)
        rank = np.int64(0)
        cap = np.int64(EMAX * 128)
        ok = True
        for e in range(E):
            p = packed[e]
            tile = (p >> 27) - tile_base
            if tile != prev_tile:
                prev_tile = tile
                rank = 0
            elif rank >= cap:
                ok = False
                continue
            pos = (rank & 127) * EB + tile * EMAX + (rank >> 7)
            ed[pos] = np.int32(((p >> 20) & 127) << 20) | np.int32(p & 0xFFFFF)
            rank += 1
        return ok

    @numba.njit(cache=True, fastmath=False)
    def _nb_edge_scatter(packed, ed, NT, EMAX, EB):
        E = packed.shape[0]
        prev_tile = np.int64(-1)
        rank = np.int64(0)
        cap = np.int64(EMAX * 128)
        ok = True
        for e in range(E):
            p = packed[e]
            tile = p >> 27
            if tile != prev_tile:
                prev_tile = tile
                rank = 0
            elif rank >= cap:
                ok = False
                continue
            core = tile // NT
            pos = ((core * 128 + (rank & 127)) * EB
                   + (tile - core * NT) * EMAX + (rank >> 7))
            ed[pos] = np.int32(((p >> 20) & 127) << 20) | np.int32(p & 0xFFFFF)
            rank += 1
        return ok

    HAVE_NUMBA = True
except Exception:
    HAVE_NUMBA = False

# ---------------- full-size problem config ----------------
FULL_CFG = dict(
    N_NODES=1_000_000,
    N_EDGES=16_000_000,
    NUM_GRAPHS=32_768,
    IN_DIM=64,
    EMB=32,
    NCLS=109,
    N_CORES=8,
    WT=33,      # tiles per 128-graph window
    EMAX=19,    # edge blocks (of 128) per node tile
    SUP=4,      # tiles per hardware-loop body (edge stage)
    HP_FP8=False,  # float8 hp table fails the 2e-2 gate (rel 2.2e-2)
    HP_INT8=True,  # int8 hp table with per-column scales (halves table bytes)
)


def _derive(cfg):
    d = dict(cfg)
    d["GRAPHS_PER"] = d["NUM_GRAPHS"] // d["N_CORES"]
    d["WPC"] = d["GRAPHS_PER"] // 128            # windows per core
    d["NT"] = d["WPC"] * d["WT"]                 # node tiles per core
    d["NSLOT_CORE"] = d["NT"] * 128
    d["EB"] = d["NT"] * d["EMAX"]                # edge blocks per core
    d["TSH"] = -(-d["N_NODES"] // d["N_CORES"])  # hp shard rows per core
    d["NSUP"] = d["NT"] // d["SUP"]
    assert d["NT"] % d["SUP"] == 0
    return d


_CACHE = {}


# ---------------- device program ----------------
def build_program(cfg):
    import concourse.bacc as bacc
    import concourse.mybir as mybir
    import concourse.tile as tile
    from concourse import bass

    ds = bass.ds
    AT = mybir.AluOpType
    FT = mybir.ActivationFunctionType
    f32 = mybir.dt.float32
    bf16 = mybir.dt.bfloat16
    i32 = mybir.dt.int32
    if cfg.get("HP_INT8"):
        hp_dt = mybir.dt.int8
    elif cfg.get("HP_FP8"):
        hp_dt = mybir.dt.float8e4
    else:
        hp_dt = bf16

    NT, WT, WPC, EMAX, SUP, NSUP = (
        cfg["NT"], cfg["WT"], cfg["WPC"], cfg["EMAX"], cfg["SUP"], cfg["NSUP"])
    EB = cfg["EB"]
    TSH = cfg["TSH"]
    NTAB = TSH * cfg["N_CORES"]
    EMB = cfg["EMB"]
    NCLS = cfg["NCLS"]
    NSLOT_CORE = cfg["NSLOT_CORE"]
    GRAPHS_PER = cfg["GRAPHS_PER"]

    nc = bacc.Bacc("TRN2", target_bir_lowering=False, debug=False,
                   num_devices=cfg["N_CORES"])

    # -------- IO --------
    inp = {}
    for s in ("s", "t"):
        inp[f"hp{s}"] = nc.dram_tensor(f"hp{s}", [TSH, EMB], hp_dt, kind="ExternalInput")
        inp[f"ed{s}"] = nc.dram_tensor(f"ed{s}", [128, EB], i32, kind="ExternalInput")
        inp[f"bl{s}"] = nc.dram_tensor(f"bl{s}", [128, NT], mybir.dt.uint8, kind="ExternalInput")
        inp[f"nid{s}"] = nc.dram_tensor(f"nid{s}", [128, NT], i32, kind="ExternalInput")
        inp[f"invc{s}"] = nc.dram_tensor(f"invc{s}", [128, WPC], f32, kind="ExternalInput")
        if cfg.get("HP_INT8"):
            inp[f"hsc{s}"] = nc.dram_tensor(f"hsc{s}", [128, EMB], f32, kind="ExternalInput")
    iota_in = nc.dram_tensor("iota", [128, 128], bf16, kind="ExternalInput")
    ident_in = nc.dram_tensor("ident", [128, 128], f32, kind="ExternalInput")
    bgr_in = nc.dram_tensor("bgr", [128, SUP * EMB], f32, kind="ExternalInput")
    wo_in = nc.dram_tensor("wo", [2 * EMB, NCLS], f32, kind="ExternalInput")
    bo_in = nc.dram_tensor("bo", [128, NCLS], f32, kind="ExternalInput")

    out = nc.dram_tensor("out", [GRAPHS_PER, NCLS], bf16, kind="ExternalOutput")

    # -------- internal DRAM --------
    hp_full = {}
    hout = {}
    for s in ("s", "t"):
        hp_full[s] = nc.dram_tensor(f"hp_full_{s}", [NTAB, EMB], hp_dt,
                                    addr_space="Shared")
        hout[s] = nc.dram_tensor(f"hout_{s}", [NSLOT_CORE, EMB], bf16)

    groups = [list(range(cfg["N_CORES"]))]

    with tile.TileContext(nc) as tc:
        with tc.tile_pool(name="const", bufs=1) as cp, \
             tc.tile_pool(name="sb", bufs=4) as sb, \
             tc.tile_pool(name="gat", bufs=3) as gp, \
             tc.tile_pool(name="ps", bufs=2, space="PSUM") as pp, \
             tc.tile_pool(name="ps2", bufs=2, space="PSUM") as pp2, \
             tc.tile_pool(name="ps3", bufs=1, space="PSUM") as pp3:

            # constants
            iota_t = cp.tile([128, 128], bf16)
            nc.sync.dma_start(out=iota_t[:], in_=iota_in[:])
            ident_t = cp.tile([128, 128], f32)
            nc.sync.dma_start(out=ident_t[:], in_=ident_in[:])
            bgr_t = cp.tile([128, SUP * EMB], f32)
            nc.sync.dma_start(out=bgr_t[:], in_=bgr_in[:])
            wo_t = cp.tile([2 * EMB, NCLS], f32)
            nc.sync.dma_start(out=wo_t[:], in_=wo_in[:])
            bo_t = cp.tile([128, NCLS], f32)
            nc.sync.dma_start(out=bo_t[:], in_=bo_in[:])
            invc_t = {}
            hsc_t = {}
            for s in ("s", "t"):
                invc_t[s] = cp.tile([128, WPC], f32, tag=f"invc{s}",
                                    name=f"invc{s}_t")
                nc.sync.dma_start(out=invc_t[s][:], in_=inp[f"invc{s}"][:])
                if cfg.get("HP_INT8"):
                    hsc_t[s] = cp.tile([128, EMB], f32, tag=f"hsc{s}",
                                       name=f"hsc{s}_t")
                    nc.sync.dma_start(out=hsc_t[s][:], in_=inp[f"hsc{s}"][:])

            # AllGather hp shards -> full tables (stage IO -> internal first;
            # collectives cannot read ExternalInput tensors)
            for s in ("s", "t"):
                stage = nc.dram_tensor(f"hp_stage_{s}", [TSH, EMB], hp_dt)
                nc.sync.dma_start(out=stage[:], in_=inp[f"hp{s}"][:])
                nc.gpsimd.collective_compute(
                    "AllGather", mybir.AluOpType.bypass,
                    replica_groups=groups,
                    ins=[stage[:]],
                    outs=[hp_full[s][:]],
                )

            # -------- edge aggregation stage --------
            for s in ("s", "t"):
                edt, nidt, hpf, hos = (
                    inp[f"ed{s}"], inp[f"nid{s}"],
                    hp_full[s], hout[s])

                with tc.For_i(0, NSUP, 1) as g:
                    ed_raw = sb.tile([128, SUP * EMAX], i32, tag="ed_raw")
                    nc.sync.dma_start(out=ed_raw[:], in_=edt[:, ds(g * (SUP * EMAX), SUP * EMAX)])
                    nid_t = sb.tile([128, SUP], i32, tag="nid")
                    nc.sync.dma_start(out=nid_t[:], in_=nidt[:, ds(g * SUP, SUP)])

                    # unpack: src = lo20, dstloc = hi
                    srcs = sb.tile([128, SUP * EMAX], i32, tag="srcs")
                    nc.vector.tensor_scalar(out=srcs[:], in0=ed_raw[:],
                                            scalar1=0xFFFFF, scalar2=None,
                                            op0=AT.bitwise_and)
                    dloc_i = sb.tile([128, SUP * EMAX], i32, tag="dloc_i")
                    nc.vector.tensor_scalar(out=dloc_i[:], in0=ed_raw[:],
                                            scalar1=20, scalar2=None,
                                            op0=AT.logical_shift_right)
                    dloc = sb.tile([128, SUP * EMAX], f32, tag="dloc")
                    nc.vector.tensor_copy(out=dloc[:], in_=dloc_i[:])

                    for u in range(SUP):
                        # self rows hp[node] for this tile (one offset/partition)
                        selfg = gp.tile([128, EMB], hp_dt, tag="selfg")
                        nc.gpsimd.indirect_dma_start(
                            out=selfg[:], out_offset=None,
                            in_=hpf[:],
                            in_offset=bass.IndirectOffsetOnAxis(
                                ap=nid_t[:, u:u + 1], axis=0))
                        self32 = sb.tile([128, EMB], f32, tag="self32")
                        nc.vector.tensor_copy(out=self32[:], in_=selfg[:])

                        agg = pp.tile([128, EMB + 1], f32, tag="agg")
                        for b in range(EMAX):
                            G = gp.tile([128, EMB + 1], hp_dt, tag="G")
                            nc.vector.memset(G[:, EMB:EMB + 1], 1)
                            nc.gpsimd.indirect_dma_start(
                                out=G[:, 0:EMB], out_offset=None,
                                in_=hpf[:],
                                in_offset=bass.IndirectOffsetOnAxis(
                                    ap=srcs[:, u * EMAX + b: u * EMAX + b + 1],
                                    axis=0))
                            if cfg.get("HP_INT8"):
                                Gb = sb.tile([128, EMB + 1], bf16, tag="Gb")
                                nc.vector.tensor_copy(out=Gb[:], in_=G[:])
                                rhs_t = Gb
                                s_dt = bf16
                            else:
                                rhs_t = G
                                s_dt = hp_dt
                            S = sb.tile([128, 128], s_dt, tag="S")
                            nc.vector.tensor_scalar(
                                out=S[:], in0=iota_t[:],
                                scalar1=dloc[:, u * EMAX + b: u * EMAX + b + 1],
                                scalar2=None, op0=AT.is_equal)
                            nc.tensor.matmul(out=agg[:], lhsT=S[:],
                                             rhs=rhs_t[:],
                                             start=(b == 0), stop=(b == EMAX - 1))
                        # dinv = 1/sqrt(count+1) from the ones-column
                        dgc = sb.tile([128, 1], f32, tag="dgc")
                        nc.vector.tensor_scalar(out=dgc[:], in0=agg[:, EMB:EMB + 1],
                                                scalar1=1.0, scalar2=None,
                                                op0=AT.add)
                        sqc = sb.tile([128, 1], f32, tag="sqc")
                        nc.scalar.activation(out=sqc[:], in_=dgc[:], func=FT.Sqrt)
                        dvc = sb.tile([128, 1], f32, tag="dvc")
                        nc.vector.reciprocal(out=dvc[:], in_=sqc[:])
                        # combine: tanh(dinv*(agg + self) + b)
                        c0 = sb.tile([128, EMB], f32, tag="c0")
                        nc.vector.tensor_tensor(out=c0[:], in0=agg[:, 0:EMB],
                                                in1=self32[:],
                                                op=AT.add)
                        if cfg.get("HP_INT8"):
                            nc.vector.tensor_tensor(out=c0[:], in0=c0[:],
                                                    in1=hsc_t[s][:],
                                                    op=AT.mult)
                        nc.vector.tensor_scalar(out=c0[:], in0=c0[:],
                                                scalar1=dvc[:], scalar2=None,
                                                op0=AT.mult)
                        nc.vector.tensor_tensor(
                            out=c0[:], in0=c0[:],
                            in1=bgr_t[:, u * EMB:(u + 1) * EMB],
                            op=AT.add)
                        th = sb.tile([128, EMB], f32, tag="th")
                        nc.scalar.activation(out=th[:], in_=c0[:], func=FT.Tanh)
                        ho = sb.tile([128, EMB], bf16, tag="ho")
                        nc.vector.tensor_copy(out=ho[:], in_=th[:])
                        nc.sync.dma_start(
                            out=hos[ds((g * SUP + u) * 128, 128), :], in_=ho[:])

            # -------- pooling + classifier stage --------
            with tc.For_i(0, WPC, 1) as w:
                embs = {}
                for s in ("s", "t"):
                    blt = inp[f"bl{s}"]
                    bl_u = sb.tile([128, WT], mybir.dt.uint8, tag="bl_u")
                    nc.sync.dma_start(out=bl_u[:], in_=blt[:, ds(w * WT, WT)])
                    bl_t = sb.tile([128, WT], f32, tag="bl")
                    nc.vector.tensor_copy(out=bl_t[:], in_=bl_u[:])
                    pps = pp2.tile([128, EMB], f32, tag="pool")
                    for i in range(WT):
                        hr = sb.tile([128, EMB], bf16, tag="hr")
                        nc.sync.dma_start(
                            out=hr[:],
                            in_=hout[s][ds((w * WT + i) * 128, 128), :])
                        S2 = sb.tile([128, 128], bf16, tag="S2")
                        nc.vector.tensor_scalar(out=S2[:], in0=iota_t[:],
                                                scalar1=bl_t[:, i:i + 1],
                                                scalar2=None, op0=AT.is_equal)
                        nc.tensor.matmul(out=pps[:], lhsT=S2[:], rhs=hr[:],
                                         start=(i == 0), stop=(i == WT - 1))
                    pooled = sb.tile([128, EMB], f32, tag="pooled")
                    nc.vector.tensor_scalar(out=pooled[:], in0=pps[:],
                                            scalar1=invc_t[s][:, ds(w, 1)],
                                            scalar2=None, op0=AT.mult)
                    emb = sb.tile([128, EMB], f32, tag=f"emb{s}")
                    nc.scalar.activation(out=emb[:], in_=pooled[:], func=FT.Tanh)
                    embs[s] = emb

                embT = sb.tile([2 * EMB, 128], f32, tag="embT")
                for s_i, s in enumerate(("s", "t")):
                    tp = pp3.tile([EMB, 128], f32, tag="tp")
                    nc.tensor.transpose(out=tp[:], in_=embs[s][:], identity=ident_t[:])
                    nc.vector.tensor_copy(out=embT[s_i * EMB:(s_i + 1) * EMB, :],
                                          in_=tp[:])
                ocol = pp3.tile([128, NCLS], f32, tag="ocol")
                nc.tensor.matmul(out=ocol[:], lhsT=embT[:], rhs=wo_t[:],
                                 start=True, stop=True)
                ob = sb.tile([128, NCLS], bf16, tag="ob")
                nc.vector.tensor_tensor(out=ob[:], in0=ocol[:],
                                        in1=bo_t[:], op=AT.add)
                nc.sync.dma_start(out=out[ds(w * 128, 128), :], in_=ob[:])

    nc.compile()
    return nc


# ---------------- host preprocessing ----------------
def _side_prep(cfg, x, edge_index, batch, W_gcn, put=None, put_piece=None,
               side="s"):
    """Per-side host prep. Returns dict of global per-core arrays. When
    `put`/`put_piece` are given, emits arrays (or per-core pieces) as soon
    as they are ready so transfers overlap with the remaining prep."""
    emit = put if put is not None else (lambda name, arr: None)
    NC = cfg["N_CORES"]
    NT, WT, WPC, EMAX = cfg["NT"], cfg["WT"], cfg["WPC"], cfg["EMAX"]
    EB = cfg["EB"]
    NSLOT_CORE = cfg["NSLOT_CORE"]
    N = cfg["N_NODES"]
    TSH = cfg["TSH"]
    NWIN = cfg["NUM_GRAPHS"] // 128          # global windows

    batch = np.ascontiguousarray(np.asarray(batch, dtype=np.int64)).astype(np.int32)
    src = np.asarray(edge_index[0]).astype(np.int32, copy=False)
    dst = np.asarray(edge_index[1]).astype(np.int32, copy=False)

    # --- slot layout ---
    Wn = (batch >> 7).astype(np.int32)               # window of each node
    wns = np.searchsorted(batch, 128 * np.arange(NWIN + 1)).astype(np.int32)
    wcounts = np.diff(wns)
    if wcounts.max() > WT * 128:
        raise RuntimeError(f"window overflow: {wcounts.max()} > {WT * 128}")
    # slot base of window W: core(W)*NSLOT_CORE + (W % WPC)*WT*128
    warr = np.arange(NWIN, dtype=np.int32)
    sbase = (warr // WPC) * NSLOT_CORE + (warr % WPC) * (WT * 128)
    slot = (np.arange(N, dtype=np.int32) - wns[Wn]) + sbase[Wn]

    # --- degrees + packed edge keys (fused when numba available) ---
    if HAVE_NUMBA:
        packed, deg = _nb_pack_deg(src, dst, slot, N)
    else:
        deg = np.bincount(dst, minlength=N).astype(np.float32)
        packed = (slot[dst].astype(np.int64) << 20) | src.astype(np.int64)
    dinv = 1.0 / np.sqrt(deg + 1.0)
    h = np.asarray(x, np.float32) @ np.asarray(W_gcn, np.float32)
    hsc_big = None
    if cfg.get("HP_INT8"):
        hp_pad = np.zeros((TSH * NC, cfg["EMB"]), np.int8)
        scales = np.empty(cfg["EMB"], np.float32)
        if HAVE_NUMBA:
            _nb_quant_i8(h, dinv, hp_pad[:N], scales)
        else:
            hpv = h * dinv[:, None]
            scales[:] = np.maximum(np.abs(hpv).max(axis=0), 1e-30) / 127.0
            hp_pad[:N] = np.clip(np.rint(hpv / scales[None, :]),
                                 -127, 127).astype(np.int8)
        hsc_big = np.ascontiguousarray(
            np.broadcast_to(scales[None, :], (NC * 128, cfg["EMB"])),
            dtype=np.float32)
        emit(f"hsc{side}", hsc_big)
    elif cfg.get("HP_FP8"):
        hp_pad = np.zeros((TSH * NC, cfg["EMB"]), ml_dtypes.float8_e4m3)
        if HAVE_NUMBA:
            _nb_scale_fp8(h, dinv, hp_pad.view(np.uint8)[:N])
        else:
            hp_pad[:N] = (h * dinv[:, None]).astype(ml_dtypes.float8_e4m3)
    else:
        hp_pad = np.zeros((TSH * NC, cfg["EMB"]), BF16)
        if HAVE_NUMBA:
            _nb_scale_bf16(h, dinv, hp_pad.view(np.uint16)[:N])
        else:
            hp_pad[:N] = (h * dinv[:, None]).astype(BF16)
    if put_piece is not None:
        for k in range(NC):
            put_piece(f"hp{side}", k, hp_pad[k * TSH:(k + 1) * TSH])
    else:
        emit(f"hp{side}", hp_pad)

    # --- per-slot arrays, laid out [core, 128 lanes, NT] ---
    core_n = slot // NSLOT_CORE
    local = slot - core_n * NSLOT_CORE
    pos_n = (core_n * 128 + (local & 127)) * NT + (local >> 7)
    bl_sl = np.full(NC * 128 * NT, 255, np.uint8)
    bl_sl[pos_n] = (batch & 127).astype(np.uint8)
    nid_sl = np.zeros(NC * 128 * NT, np.int32)
    nid_sl[pos_n] = np.arange(N, dtype=np.int32)
    emit(f"bl{side}", bl_sl.reshape(NC * 128, NT))
    emit(f"nid{side}", nid_sl.reshape(NC * 128, NT))
    cnt = np.bincount(batch, minlength=cfg["NUM_GRAPHS"]).astype(np.float32)
    invc = (1.0 / np.maximum(cnt, 1.0)).reshape(NC, WPC, 128).transpose(0, 2, 1)
    invc = np.ascontiguousarray(invc, dtype=np.float32)
    emit(f"invc{side}", invc.reshape(NC * 128, WPC))

    # --- edge sort by dst slot + scatter into padded per-tile blocks ---
    packed.sort()
    if put_piece is not None and HAVE_NUMBA:
        cuts = np.searchsorted(
            packed,
            (np.arange(NC + 1, dtype=np.int64) * NSLOT_CORE) << 20).astype(np.int64)
        ed = np.full((NC, 128 * EB), (200 << 20), np.int32)
        for k in range(NC):
            if not _nb_edge_scatter_core(packed[cuts[k]:cuts[k + 1]], ed[k],
                                         NT, EMAX, EB, k * NT):
                raise RuntimeError("tile overflow")
            put_piece(f"ed{side}", k, ed[k].reshape(128, EB))
        del packed
        return None
    ed = np.full(NC * 128 * EB, (200 << 20), np.int32)   # dummy: dloc=200, src=0
    if HAVE_NUMBA:
        if not _nb_edge_scatter(packed, ed, NT, EMAX, EB):
            raise RuntimeError("tile overflow")
        del packed
    else:
        srcs = (packed & 0xFFFFF).astype(np.int32)
        dss = (packed >> 20).astype(np.int32)
        del packed
        tile_g = dss >> 7                             # global tile id
        dloc = dss & 127
        NTILE_G = NC * NT
        tstart = np.searchsorted(dss, 128 * np.arange(NTILE_G + 1)).astype(np.int32)
        counts = np.diff(tstart)
        if counts.max() > EMAX * 128:
            raise RuntimeError(f"tile overflow: {counts.max()} > {EMAX * 128}")
        rank = np.arange(len(srcs), dtype=np.int32) - tstart[tile_g]
        core_e = tile_g // NT
        pos_e = ((core_e * 128 + (rank & 127)) * EB
                 + (tile_g - core_e * NT) * EMAX + (rank >> 7))
        ed[pos_e] = (dloc << 20) | srcs
    emit(f"ed{side}", ed.reshape(NC * 128, EB))

    r = dict(
        hp=hp_pad.reshape(NC, TSH, cfg["EMB"]),
        ed=ed.reshape(NC, 128, EB),
        bl=bl_sl.reshape(NC, 128, NT),
        nid=nid_sl.reshape(NC, 128, NT),
        invc=invc.reshape(NC, 128, WPC),
    )
    if hsc_big is not None:
        r["hsc"] = hsc_big.reshape(NC, 128, cfg["EMB"])
    return r


def _prepare_fast(nc, n_cores):
    """Build a cached jitted executable around the bass custom call so real
    calls can pass pre-sharded jax arrays (transfer overlapped with prep)."""
    import jax
    import concourse.mybir as mybir
    from concourse import bass2jax
    from jax.experimental.shard_map import shard_map
    from jax.sharding import Mesh, PartitionSpec, NamedSharding

    bass2jax.install_neuronx_cc_hook()
    assert nc.dbg_addr is None
    pid_name = (nc.partition_id_tensor.name
                if nc.partition_id_tensor is not None else None)

    in_names, out_names, out_avals, zero_shapes = [], [], [], []
    for alloc in nc.m.functions[0].allocations:
        if not isinstance(alloc, mybir.MemoryLocationSet):
            continue
        name = alloc.memorylocations[0].name
        if alloc.kind == "ExternalInput":
            if name != pid_name:
                in_names.append(name)
        elif alloc.kind == "ExternalOutput":
            out_names.append(name)
            shape = tuple(alloc.tensor_shape)
            dtype = mybir.dt.np(alloc.dtype)
            out_avals.append(jax.core.ShapedArray(shape, dtype))
            zero_shapes.append((shape, dtype))
    n_params = len(in_names)
    all_names = in_names + out_names
    if pid_name is not None:
        all_names = all_names + [pid_name]
    donate = tuple(range(n_params, n_params + len(out_names)))

    def _body(*args):
        operands = list(args)
        if pid_name is not None:
            operands.append(bass2jax.partition_id_tensor())
        outs = bass2jax._bass_exec_p.bind(
            *operands,
            out_avals=tuple(out_avals),
            in_names=tuple(all_names),
            out_names=tuple(out_names),
            lowering_input_output_aliases=(),
            sim_require_finite=True,
            sim_require_nnan=True,
            nc=nc,
        )
        return tuple(outs)

    devices = jax.devices()[:n_cores]
    mesh = Mesh(np.asarray(devices), ("core",))
    nspecs = n_params + len(out_names)
    sharded = jax.jit(
        shard_map(_body, mesh=mesh,
                  in_specs=(PartitionSpec("core"),) * nspecs,
                  out_specs=(PartitionSpec("core"),) * len(out_names),
                  check_rep=False),
        donate_argnums=donate, keep_unused=True)
    sh = NamedSharding(mesh, PartitionSpec("core"))
    return dict(sharded=sharded, in_names=in_names, out_names=out_names,
                zero_shapes=zero_shapes, sh=sh, n_cores=n_cores,
                device_put=jax.device_put, devices=devices,
                make_array=jax.make_array_from_single_device_arrays)


def _fast_exec(fast, arrays):
    """arrays: dict name -> jax array (pre-sharded) or np array (global
    concat layout). Returns list of np outputs in out_names order."""
    put = fast["device_put"]
    ops = []
    for name in fast["in_names"]:
        a = arrays[name]
        if isinstance(a, np.ndarray):
            a = put(a, fast["sh"])
        ops.append(a)
    zeros = [put(np.zeros((fast["n_cores"] * s[0], *s[1:]), d), fast["sh"])
             for (s, d) in fast["zero_shapes"]]
    outs = fast["sharded"](*ops, *zeros)
    return [np.asarray(o) for o in outs]


def run(cfg, nc, x_s, edge_index_s, x_s_batch, x_t, edge_index_t, x_t_batch, y,
        W_gcn, b_gcn, W_out, b_out):
    from concourse import bass_utils

    NC = cfg["N_CORES"]
    ps = _side_prep(cfg, x_s, edge_index_s, x_s_batch, W_gcn)
    pt = _side_prep(cfg, x_t, edge_index_t, x_t_batch, W_gcn)

    iota = np.broadcast_to(np.arange(128, dtype=np.float32).astype(BF16),
                           (128, 128)).copy()
    ident = np.eye(128, dtype=np.float32)
    bgr = np.broadcast_to(np.tile(np.asarray(b_gcn, np.float32), cfg["SUP"]),
                          (128, cfg["SUP"] * cfg["EMB"])).copy()
    wo = np.asarray(W_out, np.float32)
    bo = np.broadcast_to(np.asarray(b_out, np.float32),
                         (128, len(np.asarray(b_out)))).copy()

    in_maps = []
    for k in range(NC):
        m = {
            "iota": iota, "ident": ident, "bgr": bgr, "wo": wo, "bo": bo,
        }
        for s, p in (("s", ps), ("t", pt)):
            m[f"hp{s}"] = p["hp"][k]
            m[f"ed{s}"] = p["ed"][k]
            m[f"bl{s}"] = p["bl"][k]
            if "hsc" in p:
                m[f"hsc{s}"] = p["hsc"][k]
            m[f"nid{s}"] = p["nid"][k]
            m[f"invc{s}"] = p["invc"][k]
        in_maps.append(m)

    res = bass_utils.run_bass_kernel_spmd(nc, in_maps, core_ids=list(range(NC)))
    return np.concatenate([res.results[k]["out"] for k in range(NC)],
                          axis=0).astype(np.float32)


def _const_arrays(cfg, b_gcn, W_out, b_out):
    NC = cfg["N_CORES"]
    iota = np.broadcast_to(np.arange(128, dtype=np.float32).astype(BF16),
                           (128, 128)).copy()
    ident = np.eye(128, dtype=np.float32)
    bgr = np.broadcast_to(np.tile(np.asarray(b_gcn, np.float32), cfg["SUP"]),
                          (128, cfg["SUP"] * cfg["EMB"])).copy()
    wo = np.asarray(W_out, np.float32)
    bo = np.broadcast_to(np.asarray(b_out, np.float32),
                         (128, len(np.asarray(b_out)))).copy()
    return {
        "iota": np.tile(iota, (NC, 1)),
        "ident": np.tile(ident, (NC, 1)),
        "bgr": np.tile(bgr, (NC, 1)),
        "wo": np.tile(wo, (NC, 1)),
        "bo": np.tile(bo, (NC, 1)),
    }


def run_fast(cfg, fast, x_s, edge_index_s, x_s_batch, x_t, edge_index_t,
             x_t_batch, y, W_gcn, b_gcn, W_out, b_out):
    import os
    import time
    prof = bool(os.environ.get("KERNEL_PROF"))
    t0 = time.perf_counter()
    arrays = {}
    pieces = {}
    dput = fast["device_put"]
    devices = fast["devices"]

    def put(name, arr):
        arrays[name] = dput(arr, fast["sh"])

    def put_piece(name, k, piece):
        pieces.setdefault(name, {})[k] = dput(piece, devices[k])

    # donated output zero-buffers don't depend on inputs: ship them first
    zeros = [dput(np.zeros((fast["n_cores"] * s[0], *s[1:]), d), fast["sh"])
             for (s, d) in fast["zero_shapes"]]
    for name, arr in _const_arrays(cfg, b_gcn, W_out, b_out).items():
        put(name, arr)
    _side_prep(cfg, x_s, edge_index_s, x_s_batch, W_gcn, put=put,
               put_piece=put_piece, side="s")
    if prof:
        print(f"  prep s done: {time.perf_counter()-t0:.2f}s", flush=True)
    _side_prep(cfg, x_t, edge_index_t, x_t_batch, W_gcn, put=put,
               put_piece=put_piece, side="t")
    if prof:
        print(f"  prep t done: {time.perf_counter()-t0:.2f}s", flush=True)
    try:
        for name, pc in pieces.items():
            shards = [pc[k] for k in range(fast["n_cores"])]
            gshape = (sum(s.shape[0] for s in shards),) + shards[0].shape[1:]
            arrays[name] = fast["make_array"](gshape, fast["sh"], shards)
        ops = [arrays[n] for n in fast["in_names"]]
        outs = fast["sharded"](*ops, *zeros)
        outs = [np.asarray(o).astype(np.float32) for o in outs]
        if prof:
            print(f"  exec done:   {time.perf_counter()-t0:.2f}s", flush=True)
    finally:
        for a in arrays.values():
            try:
                a.delete()
            except Exception:
                pass
    return outs[0]


def kernel(x_s, edge_index_s, x_s_batch, x_t, edge_index_t, x_t_batch, y,
           W_gcn, b_gcn, W_out, b_out):
    cfg = _CACHE.get("cfg")
    if cfg is None:
        cfg = _derive(FULL_CFG)
        _CACHE["cfg"] = cfg
    nc = _CACHE.get("nc")
    if nc is None:
        nc = build_program(cfg)
        _CACHE["nc"] = nc
    fast = _CACHE.get("fast")
    if fast is not None:
        try:
            return run_fast(cfg, fast, x_s, edge_index_s, x_s_batch, x_t,
                            edge_index_t, x_t_batch, y, W_gcn, b_gcn, W_out,
                            b_out)
        except Exception:
            pass
    return run(cfg, nc, x_s, edge_index_s, x_s_batch, x_t, edge_index_t,
               x_t_batch, y, W_gcn, b_gcn, W_out, b_out)


def _warmup():
    """Compile the PJRT executable (and warm transfer paths) with dummy
    zero inputs so the first real kernel() call doesn't pay for it."""
    cfg = _CACHE["cfg"]
    fast = _CACHE["fast"]
    NC = cfg["N_CORES"]
    NT, WPC, EB, TSH = cfg["NT"], cfg["WPC"], cfg["EB"], cfg["TSH"]
    m = {
        "iota": np.zeros((NC * 128, 128), BF16),
        "ident": np.zeros((NC * 128, 128), np.float32),
        "bgr": np.zeros((NC * 128, cfg["SUP"] * cfg["EMB"]), np.float32),
        "wo": np.zeros((NC * 2 * cfg["EMB"], cfg["NCLS"]), np.float32),
        "bo": np.zeros((NC * 128, cfg["NCLS"]), np.float32),
    }
    if cfg.get("HP_INT8"):
        hp_np = np.int8
    elif cfg.get("HP_FP8"):
        hp_np = ml_dtypes.float8_e4m3
    else:
        hp_np = BF16
    for s in ("s", "t"):
        m[f"hp{s}"] = np.zeros((NC * TSH, cfg["EMB"]), hp_np)
        if cfg.get("HP_INT8"):
            m[f"hsc{s}"] = np.zeros((NC * 128, cfg["EMB"]), np.float32)
        m[f"ed{s}"] = np.zeros((NC * 128, EB), np.int32)
        m[f"bl{s}"] = np.zeros((NC * 128, NT), np.uint8)
        m[f"nid{s}"] = np.zeros((NC * 128, NT), np.int32)
        m[f"invc{s}"] = np.zeros((NC * 128, WPC), np.float32)
    _fast_exec(fast, m)


# compile at import so the first kernel() call doesn't pay for it
try:
    import os as _os
    if HAVE_NUMBA:
        _s = np.zeros(4, np.int32)
        _sl = np.zeros(4, np.int32)
        _p, _ = _nb_pack_deg(_s, _s, _sl, 4)
        _p.sort()
        _nb_edge_scatter(_p, np.zeros(128 * 19, np.int32), 1, 19, 19)
        _nb_edge_scatter_core(_p, np.zeros(128 * 19, np.int32), 1, 19, 19, 0)
        _nb_scale_bf16(np.zeros((2, 2), np.float32), np.ones(2, np.float32),
                       np.zeros((2, 2), np.uint16))
        _nb_scale_fp8(np.zeros((2, 2), np.float32), np.ones(2, np.float32),
                      np.zeros((2, 2), np.uint8))
        _nb_quant_i8(np.ones((2, 2), np.float32), np.ones(2, np.float32),
                     np.zeros((2, 2), np.int8), np.zeros(2, np.float32))
    _CACHE["cfg"] = _derive(FULL_CFG)
    _CACHE["nc"] = build_program(_CACHE["cfg"])
    try:
        _CACHE["fast"] = _prepare_fast(_CACHE["nc"], _CACHE["cfg"]["N_CORES"])
        if not _os.environ.get("KERNEL_NO_WARMUP"):
            _warmup()
    except Exception:
        _CACHE.pop("fast", None)
except Exception:
    _CACHE.pop("nc", None)


# revision 41
# speedup vs baseline: 1.1759x; 1.1759x over previous
"""GCN MixturePredictor kernel for 8 Trainium2 NeuronCores.

Strategy (everything heavy on device; host only sorts/indexes):
- Graphs sharded 4096/core. Node ids re-laid into padded "slots": per core
  32 windows (128 graphs each) x WT tiles x 128 slots, so pooling windows are
  tile-aligned and fully static.
- hp = (x @ W_gcn) * rsqrt(deg+1) computed on host (tiny GEMM + fused numba
  bf16 cast), sharded 1/8 per core, AllGathered on device to a 1M-row table.
- Edges sorted by destination slot with a packed int64 np.sort (radix, fast),
  then scattered (numba, sequential ranks) into per-tile padded blocks of
  EMAX x 128. Each edge is packed as (dstloc << 20) | src into int32;
  dummy slots carry dstloc=200 so their one-hot row is all-zero.
- Device per 128-node tile: per 128-edge block one indirect DMA gathers
  hp[src] rows into a [128, 33] tile (col 32 preset to 1.0); one-hot
  matrices (is_equal vs an iota row) feed EMAX accumulating bf16 matmuls
  into PSUM. Column 32 of the accumulator yields the in-degree, from which
  dinv = 1/sqrt(deg+1) is computed on device; combine with the gathered
  self row, add bias, tanh -> hout (bf16).
- Pooling per 128-graph window via the same one-hot matmul over its WT
  tiles (dummy slots have bl=255 and never match); classifier matmul on
  device. Output [32768, 109] (bf16 on the wire, f32 returned).
- kernel() uses a cached jitted executable and device_puts each global array as
  soon as host prep produces it, overlapping transfer with prep; falls back
  to bass_utils.run_bass_kernel_spmd if that path is unavailable.
"""

import numpy as np
import ml_dtypes

BF16 = ml_dtypes.bfloat16

try:
    import numba

    @numba.njit(cache=True, fastmath=False)
    def _nb_pack_deg(src, dst, slot, n_nodes):
        E = src.shape[0]
        packed = np.empty(E + n_nodes, np.int64)
        deg = np.zeros(n_nodes, np.float32)
        for e in range(E):
            d = dst[e]
            deg[d] += 1.0
            packed[e] = (np.int64(slot[d]) << 20) | np.int64(src[e])
        for n in range(n_nodes):
            packed[E + n] = (np.int64(slot[n]) << 20) | np.int64(n)
        return packed, deg

    @numba.njit(cache=True, fastmath=False)
    def _nb_scale_bf16(h, dinv, out_u16):
        n, c = h.shape
        buf = np.empty(c, np.float32)
        for i in range(n):
            dv = dinv[i]
            for j in range(c):
                buf[j] = h[i, j] * dv
            bu = buf.view(np.uint32)
            for j in range(c):
                b = bu[j]
                out_u16[i, j] = np.uint16(
                    (b + np.uint32(0x7FFF) + ((b >> np.uint32(16)) & np.uint32(1)))
                    >> np.uint32(16))

    @numba.njit(cache=True, fastmath=False)
    def _nb_scale_fp8(h, dinv, out_u8):
        n, c = h.shape
        buf = np.empty(c, np.float32)
        for i in range(n):
            dv = dinv[i]
            for j in range(c):
                buf[j] = h[i, j] * dv
            bu = buf.view(np.uint32)
            for j in range(c):
                b = bu[j]
                s = np.uint8((b >> np.uint32(24)) & np.uint32(0x80))
                e = np.int64((b >> np.uint32(23)) & np.uint32(0xFF))
                m = np.int64(b & np.uint32(0x7FFFFF))
                te = e - 120          # biased target exponent (bias 7)
                if e == 0 or te < -3:
                    out_u8[i, j] = s
                elif te <= 0:
                    full = m | 0x800000
                    shift = 20 + 1 - te
                    half = np.int64(1) << (shift - 1)
                    r = (full + (half - 1) + ((full >> shift) & 1)) >> shift
                    out_u8[i, j] = s | np.uint8(r)
                elif te >= 15:
                    out_u8[i, j] = s | np.uint8(0x77)   # saturate (never hit)
                else:
                    r = (m + 0x7FFFF + ((m >> 20) & 1)) >> 20
                    out_u8[i, j] = s | np.uint8((te << 3) + r)

    @numba.njit(cache=True, fastmath=False)
    def _nb_quant_i8(h, dinv, out_i8, scales):
        n, c = h.shape
        for j in range(c):
            scales[j] = 0.0
        for i in range(n):
            dv = dinv[i]
            for j in range(c):
                v = abs(h[i, j] * dv)
                if v > scales[j]:
                    scales[j] = v
        inv = np.empty(c, np.float32)
        for j in range(c):
            if scales[j] <= 0.0:
                scales[j] = 1.0
            scales[j] = scales[j] / 127.0
            inv[j] = 1.0 / scales[j]
        for i in range(n):
            dv = dinv[i]
            for j in range(c):
                q = np.int32(np.floor(h[i, j] * dv * inv[j] + 0.5))
                if q > 127:
                    q = 127
                elif q < -127:
                    q = -127
                out_i8[i, j] = np.int8(q)

    @numba.njit(cache=True, fastmath=False)
    def _nb_edge_scatter_core(packed, ed, NT, EMAX, EB, tile_base):
        E = packed.shape[0]
        prev_tile = np.int64(-1)
        rank = np.int64(0)
        cap = np.int64(EMAX * 128)
        ok = True
        for e in range(E):
            p = packed[e]
            tile = (p >> 27) - tile_base
            if tile != prev_tile:
                prev_tile = tile
                rank = 0
            elif rank >= cap:
                ok = False
                continue
            pos = (rank & 127) * EB + tile * EMAX + (rank >> 7)
            ed[pos] = np.int32(((p >> 20) & 127) << 20) | np.int32(p & 0xFFFFF)
            rank += 1
        return ok

    @numba.njit(cache=True, fastmath=False)
    def _nb_edge_scatter(packed, ed, NT, EMAX, EB):
        E = packed.shape[0]
        prev_tile = np.int64(-1)
        rank = np.int64(0)
        cap = np.int64(EMAX * 128)
        ok = True
        for e in range(E):
            p = packed[e]
            tile = p >> 27
            if tile != prev_tile:
                prev_tile = tile
                rank = 0
            elif rank >= cap:
                ok = False
                continue
            core = tile // NT
            pos = ((core * 128 + (rank & 127)) * EB
                   + (tile - core * NT) * EMAX + (rank >> 7))
            ed[pos] = np.int32(((p >> 20) & 127) << 20) | np.int32(p & 0xFFFFF)
            rank += 1
        return ok

    HAVE_NUMBA = True
except Exception:
    HAVE_NUMBA = False

# ---------------- full-size problem config ----------------
FULL_CFG = dict(
    N_NODES=1_000_000,
    N_EDGES=16_000_000,
    NUM_GRAPHS=32_768,
    IN_DIM=64,
    EMB=32,
    NCLS=109,
    N_CORES=8,
    WT=33,      # tiles per 128-graph window
    EMAX=19,    # edge blocks (of 128) per node tile
    SUP=4,      # tiles per hardware-loop body (edge stage)
    HP_FP8=False,  # float8 hp table fails the 2e-2 gate (rel 2.2e-2)
    HP_INT8=True,  # int8 hp table with per-column scales (halves table bytes)
)


def _derive(cfg):
    d = dict(cfg)
    d["GRAPHS_PER"] = d["NUM_GRAPHS"] // d["N_CORES"]
    d["WPC"] = d["GRAPHS_PER"] // 128            # windows per core
    d["NT"] = d["WPC"] * d["WT"]                 # node tiles per core
    d["NSLOT_CORE"] = d["NT"] * 128
    d["EB"] = d["NT"] * d["EMAX"]                # edge blocks per core
    d["TSH"] = -(-d["N_NODES"] // d["N_CORES"])  # hp shard rows per core
    d["NSUP"] = d["NT"] // d["SUP"]
    assert d["NT"] % d["SUP"] == 0
    return d


_CACHE = {}


# ---------------- device program ----------------
def build_program(cfg):
    import concourse.bacc as bacc
    import concourse.mybir as mybir
    import concourse.tile as tile
    from concourse import bass

    ds = bass.ds
    AT = mybir.AluOpType
    FT = mybir.ActivationFunctionType
    f32 = mybir.dt.float32
    bf16 = mybir.dt.bfloat16
    i32 = mybir.dt.int32
    if cfg.get("HP_INT8"):
        hp_dt = mybir.dt.int8
    elif cfg.get("HP_FP8"):
        hp_dt = mybir.dt.float8e4
    else:
        hp_dt = bf16

    NT, WT, WPC, EMAX, SUP, NSUP = (
        cfg["NT"], cfg["WT"], cfg["WPC"], cfg["EMAX"], cfg["SUP"], cfg["NSUP"])
    EB = cfg["EB"]
    TSH = cfg["TSH"]
    NTAB = TSH * cfg["N_CORES"]
    EMB = cfg["EMB"]
    NCLS = cfg["NCLS"]
    NSLOT_CORE = cfg["NSLOT_CORE"]
    GRAPHS_PER = cfg["GRAPHS_PER"]

    nc = bacc.Bacc("TRN2", target_bir_lowering=False, debug=False,
                   num_devices=cfg["N_CORES"])

    # -------- IO --------
    inp = {}
    for s in ("s", "t"):
        inp[f"hp{s}"] = nc.dram_tensor(f"hp{s}", [TSH, EMB], hp_dt, kind="ExternalInput")
        inp[f"ed{s}"] = nc.dram_tensor(f"ed{s}", [128, EB], i32, kind="ExternalInput")
        inp[f"bl{s}"] = nc.dram_tensor(f"bl{s}", [128, NT], mybir.dt.uint8, kind="ExternalInput")
        inp[f"invc{s}"] = nc.dram_tensor(f"invc{s}", [128, WPC], f32, kind="ExternalInput")
        if cfg.get("HP_INT8"):
            inp[f"hsc{s}"] = nc.dram_tensor(f"hsc{s}", [128, EMB], f32, kind="ExternalInput")
    iota_in = nc.dram_tensor("iota", [128, 128], bf16, kind="ExternalInput")
    ident_in = nc.dram_tensor("ident", [128, 128], f32, kind="ExternalInput")
    bgr_in = nc.dram_tensor("bgr", [128, SUP * EMB], f32, kind="ExternalInput")
    wo_in = nc.dram_tensor("wo", [2 * EMB, NCLS], f32, kind="ExternalInput")
    bo_in = nc.dram_tensor("bo", [128, NCLS], f32, kind="ExternalInput")

    out = nc.dram_tensor("out", [GRAPHS_PER, NCLS], bf16, kind="ExternalOutput")

    # -------- internal DRAM --------
    hp_full = {}
    hout = {}
    for s in ("s", "t"):
        hp_full[s] = nc.dram_tensor(f"hp_full_{s}", [NTAB, EMB], hp_dt,
                                    addr_space="Shared")
        hout[s] = nc.dram_tensor(f"hout_{s}", [NSLOT_CORE, EMB], bf16)

    groups = [list(range(cfg["N_CORES"]))]

    with tile.TileContext(nc) as tc:
        with tc.tile_pool(name="const", bufs=1) as cp, \
             tc.tile_pool(name="sb", bufs=4) as sb, \
             tc.tile_pool(name="gat", bufs=3) as gp, \
             tc.tile_pool(name="ps", bufs=2, space="PSUM") as pp, \
             tc.tile_pool(name="ps2", bufs=2, space="PSUM") as pp2, \
             tc.tile_pool(name="ps3", bufs=1, space="PSUM") as pp3:

            # constants
            iota_t = cp.tile([128, 128], bf16)
            nc.sync.dma_start(out=iota_t[:], in_=iota_in[:])
            ident_t = cp.tile([128, 128], f32)
            nc.sync.dma_start(out=ident_t[:], in_=ident_in[:])
            bgr_t = cp.tile([128, SUP * EMB], f32)
            nc.sync.dma_start(out=bgr_t[:], in_=bgr_in[:])
            wo_t = cp.tile([2 * EMB, NCLS], f32)
            nc.sync.dma_start(out=wo_t[:], in_=wo_in[:])
            bo_t = cp.tile([128, NCLS], f32)
            nc.sync.dma_start(out=bo_t[:], in_=bo_in[:])
            invc_t = {}
            hsc_t = {}
            for s in ("s", "t"):
                invc_t[s] = cp.tile([128, WPC], f32, tag=f"invc{s}",
                                    name=f"invc{s}_t")
                nc.sync.dma_start(out=invc_t[s][:], in_=inp[f"invc{s}"][:])
                if cfg.get("HP_INT8"):
                    hsc_t[s] = cp.tile([128, EMB], f32, tag=f"hsc{s}",
                                       name=f"hsc{s}_t")
                    nc.sync.dma_start(out=hsc_t[s][:], in_=inp[f"hsc{s}"][:])

            # AllGather hp shards -> full tables (stage IO -> internal first;
            # collectives cannot read ExternalInput tensors)
            for s in ("s", "t"):
                stage = nc.dram_tensor(f"hp_stage_{s}", [TSH, EMB], hp_dt)
                nc.sync.dma_start(out=stage[:], in_=inp[f"hp{s}"][:])
                nc.gpsimd.collective_compute(
                    "AllGather", mybir.AluOpType.bypass,
                    replica_groups=groups,
                    ins=[stage[:]],
                    outs=[hp_full[s][:]],
                )

            # -------- edge aggregation stage --------
            for s in ("s", "t"):
                edt, hpf, hos = (inp[f"ed{s}"], hp_full[s], hout[s])

                with tc.For_i(0, NSUP, 1) as g:
                    ed_raw = sb.tile([128, SUP * EMAX], i32, tag="ed_raw")
                    nc.sync.dma_start(out=ed_raw[:], in_=edt[:, ds(g * (SUP * EMAX), SUP * EMAX)])

                    # unpack: src = lo20, dstloc = hi
                    srcs = sb.tile([128, SUP * EMAX], i32, tag="srcs")
                    nc.vector.tensor_scalar(out=srcs[:], in0=ed_raw[:],
                                            scalar1=0xFFFFF, scalar2=None,
                                            op0=AT.bitwise_and)
                    dloc_i = sb.tile([128, SUP * EMAX], i32, tag="dloc_i")
                    nc.vector.tensor_scalar(out=dloc_i[:], in0=ed_raw[:],
                                            scalar1=20, scalar2=None,
                                            op0=AT.logical_shift_right)
                    dloc = sb.tile([128, SUP * EMAX], f32, tag="dloc")
                    nc.vector.tensor_copy(out=dloc[:], in_=dloc_i[:])

                    for u in range(SUP):
                        # self loops are pre-appended to the edge list; the
                        # ones-column therefore accumulates deg+1 directly
                        agg = pp.tile([128, EMB + 1], f32, tag="agg")
                        for b in range(EMAX):
                            G = gp.tile([128, EMB + 1], hp_dt, tag="G")
                            nc.vector.memset(G[:, EMB:EMB + 1], 1)
                            nc.gpsimd.indirect_dma_start(
                                out=G[:, 0:EMB], out_offset=None,
                                in_=hpf[:],
                                in_offset=bass.IndirectOffsetOnAxis(
                                    ap=srcs[:, u * EMAX + b: u * EMAX + b + 1],
                                    axis=0))
                            if cfg.get("HP_INT8"):
                                Gb = sb.tile([128, EMB + 1], bf16, tag="Gb")
                                nc.vector.tensor_copy(out=Gb[:], in_=G[:])
                                rhs_t = Gb
                                s_dt = bf16
                            else:
                                rhs_t = G
                                s_dt = hp_dt
                            S = sb.tile([128, 128], s_dt, tag="S")
                            nc.vector.tensor_scalar(
                                out=S[:], in0=iota_t[:],
                                scalar1=dloc[:, u * EMAX + b: u * EMAX + b + 1],
                                scalar2=None, op0=AT.is_equal)
                            nc.tensor.matmul(out=agg[:], lhsT=S[:],
                                             rhs=rhs_t[:],
                                             start=(b == 0), stop=(b == EMAX - 1))
                        # dinv = 1/sqrt(max(count,1)); count = deg+1 via the
                        # self edges, 0 only on padded slots
                        dgc = sb.tile([128, 1], f32, tag="dgc")
                        nc.vector.tensor_scalar(out=dgc[:], in0=agg[:, EMB:EMB + 1],
                                                scalar1=1.0, scalar2=None,
                                                op0=AT.max)
                        sqc = sb.tile([128, 1], f32, tag="sqc")
                        nc.scalar.activation(out=sqc[:], in_=dgc[:], func=FT.Sqrt)
                        dvc = sb.tile([128, 1], f32, tag="dvc")
                        nc.vector.reciprocal(out=dvc[:], in_=sqc[:])
                        # combine: tanh(dinv*agg*scale + b)
                        c0 = sb.tile([128, EMB], f32, tag="c0")
                        if cfg.get("HP_INT8"):
                            nc.vector.tensor_tensor(out=c0[:], in0=agg[:, 0:EMB],
                                                    in1=hsc_t[s][:],
                                                    op=AT.mult)
                        else:
                            nc.vector.tensor_copy(out=c0[:], in_=agg[:, 0:EMB])
                        nc.vector.tensor_scalar(out=c0[:], in0=c0[:],
                                                scalar1=dvc[:], scalar2=None,
                                                op0=AT.mult)
                        nc.vector.tensor_tensor(
                            out=c0[:], in0=c0[:],
                            in1=bgr_t[:, u * EMB:(u + 1) * EMB],
                            op=AT.add)
                        th = sb.tile([128, EMB], f32, tag="th")
                        nc.scalar.activation(out=th[:], in_=c0[:], func=FT.Tanh)
                        ho = sb.tile([128, EMB], bf16, tag="ho")
                        nc.vector.tensor_copy(out=ho[:], in_=th[:])
                        nc.sync.dma_start(
                            out=hos[ds((g * SUP + u) * 128, 128), :], in_=ho[:])

            # -------- pooling + classifier stage --------
            with tc.For_i(0, WPC, 1) as w:
                embs = {}
                for s in ("s", "t"):
                    blt = inp[f"bl{s}"]
                    bl_u = sb.tile([128, WT], mybir.dt.uint8, tag="bl_u")
                    nc.sync.dma_start(out=bl_u[:], in_=blt[:, ds(w * WT, WT)])
                    bl_t = sb.tile([128, WT], f32, tag="bl")
                    nc.vector.tensor_copy(out=bl_t[:], in_=bl_u[:])
                    pps = pp2.tile([128, EMB], f32, tag="pool")
                    for i in range(WT):
                        hr = sb.tile([128, EMB], bf16, tag="hr")
                        nc.sync.dma_start(
                            out=hr[:],
                            in_=hout[s][ds((w * WT + i) * 128, 128), :])
                        S2 = sb.tile([128, 128], bf16, tag="S2")
                        nc.vector.tensor_scalar(out=S2[:], in0=iota_t[:],
                                                scalar1=bl_t[:, i:i + 1],
                                                scalar2=None, op0=AT.is_equal)
                        nc.tensor.matmul(out=pps[:], lhsT=S2[:], rhs=hr[:],
                                         start=(i == 0), stop=(i == WT - 1))
                    pooled = sb.tile([128, EMB], f32, tag="pooled")
                    nc.vector.tensor_scalar(out=pooled[:], in0=pps[:],
                                            scalar1=invc_t[s][:, ds(w, 1)],
                                            scalar2=None, op0=AT.mult)
                    emb = sb.tile([128, EMB], f32, tag=f"emb{s}")
                    nc.scalar.activation(out=emb[:], in_=pooled[:], func=FT.Tanh)
                    embs[s] = emb

                embT = sb.tile([2 * EMB, 128], f32, tag="embT")
                for s_i, s in enumerate(("s", "t")):
                    tp = pp3.tile([EMB, 128], f32, tag="tp")
                    nc.tensor.transpose(out=tp[:], in_=embs[s][:], identity=ident_t[:])
                    nc.vector.tensor_copy(out=embT[s_i * EMB:(s_i + 1) * EMB, :],
                                          in_=tp[:])
                ocol = pp3.tile([128, NCLS], f32, tag="ocol")
                nc.tensor.matmul(out=ocol[:], lhsT=embT[:], rhs=wo_t[:],
                                 start=True, stop=True)
                ob = sb.tile([128, NCLS], bf16, tag="ob")
                nc.vector.tensor_tensor(out=ob[:], in0=ocol[:],
                                        in1=bo_t[:], op=AT.add)
                nc.sync.dma_start(out=out[ds(w * 128, 128), :], in_=ob[:])

    nc.compile()
    return nc


# ---------------- host preprocessing ----------------
def _side_prep(cfg, x, edge_index, batch, W_gcn, put=None, put_piece=None,
               side="s"):
    """Per-side host prep. Returns dict of global per-core arrays. When
    `put`/`put_piece` are given, emits arrays (or per-core pieces) as soon
    as they are ready so transfers overlap with the remaining prep."""
    emit = put if put is not None else (lambda name, arr: None)
    NC = cfg["N_CORES"]
    NT, WT, WPC, EMAX = cfg["NT"], cfg["WT"], cfg["WPC"], cfg["EMAX"]
    EB = cfg["EB"]
    NSLOT_CORE = cfg["NSLOT_CORE"]
    N = cfg["N_NODES"]
    TSH = cfg["TSH"]
    NWIN = cfg["NUM_GRAPHS"] // 128          # global windows

    batch = np.ascontiguousarray(np.asarray(batch, dtype=np.int64)).astype(np.int32)
    src = np.asarray(edge_index[0]).astype(np.int32, copy=False)
    dst = np.asarray(edge_index[1]).astype(np.int32, copy=False)

    # --- slot layout ---
    Wn = (batch >> 7).astype(np.int32)               # window of each node
    wns = np.searchsorted(batch, 128 * np.arange(NWIN + 1)).astype(np.int32)
    wcounts = np.diff(wns)
    if wcounts.max() > WT * 128:
        raise RuntimeError(f"window overflow: {wcounts.max()} > {WT * 128}")
    # slot base of window W: core(W)*NSLOT_CORE + (W % WPC)*WT*128
    warr = np.arange(NWIN, dtype=np.int32)
    sbase = (warr // WPC) * NSLOT_CORE + (warr % WPC) * (WT * 128)
    slot = (np.arange(N, dtype=np.int32) - wns[Wn]) + sbase[Wn]

    # --- degrees + packed edge keys (fused when numba available) ---
    if HAVE_NUMBA:
        packed, deg = _nb_pack_deg(src, dst, slot, N)
    else:
        deg = np.bincount(dst, minlength=N).astype(np.float32)
        packed = np.concatenate([
            (slot[dst].astype(np.int64) << 20) | src.astype(np.int64),
            (slot.astype(np.int64) << 20) | np.arange(N, dtype=np.int64)])
    dinv = 1.0 / np.sqrt(deg + 1.0)
    h = np.asarray(x, np.float32) @ np.asarray(W_gcn, np.float32)
    hsc_big = None
    if cfg.get("HP_INT8"):
        hp_pad = np.zeros((TSH * NC, cfg["EMB"]), np.int8)
        scales = np.empty(cfg["EMB"], np.float32)
        if HAVE_NUMBA:
            _nb_quant_i8(h, dinv, hp_pad[:N], scales)
        else:
            hpv = h * dinv[:, None]
            scales[:] = np.maximum(np.abs(hpv).max(axis=0), 1e-30) / 127.0
            hp_pad[:N] = np.clip(np.rint(hpv / scales[None, :]),
                                 -127, 127).astype(np.int8)
        hsc_big = np.ascontiguousarray(
            np.broadcast_to(scales[None, :], (NC * 128, cfg["EMB"])),
            dtype=np.float32)
        emit(f"hsc{side}", hsc_big)
    elif cfg.get("HP_FP8"):
        hp_pad = np.zeros((TSH * NC, cfg["EMB"]), ml_dtypes.float8_e4m3)
        if HAVE_NUMBA:
            _nb_scale_fp8(h, dinv, hp_pad.view(np.uint8)[:N])
        else:
            hp_pad[:N] = (h * dinv[:, None]).astype(ml_dtypes.float8_e4m3)
    else:
        hp_pad = np.zeros((TSH * NC, cfg["EMB"]), BF16)
        if HAVE_NUMBA:
            _nb_scale_bf16(h, dinv, hp_pad.view(np.uint16)[:N])
        else:
            hp_pad[:N] = (h * dinv[:, None]).astype(BF16)
    if put_piece is not None:
        for k in range(NC):
            put_piece(f"hp{side}", k, hp_pad[k * TSH:(k + 1) * TSH])
    else:
        emit(f"hp{side}", hp_pad)

    # --- per-slot arrays, laid out [core, 128 lanes, NT] ---
    core_n = slot // NSLOT_CORE
    local = slot - core_n * NSLOT_CORE
    pos_n = (core_n * 128 + (local & 127)) * NT + (local >> 7)
    bl_sl = np.full(NC * 128 * NT, 255, np.uint8)
    bl_sl[pos_n] = (batch & 127).astype(np.uint8)
    emit(f"bl{side}", bl_sl.reshape(NC * 128, NT))
    cnt = np.bincount(batch, minlength=cfg["NUM_GRAPHS"]).astype(np.float32)
    invc = (1.0 / np.maximum(cnt, 1.0)).reshape(NC, WPC, 128).transpose(0, 2, 1)
    invc = np.ascontiguousarray(invc, dtype=np.float32)
    emit(f"invc{side}", invc.reshape(NC * 128, WPC))

    # --- edge sort by dst slot + scatter into padded per-tile blocks ---
    packed.sort()
    if put_piece is not None and HAVE_NUMBA:
        cuts = np.searchsorted(
            packed,
            (np.arange(NC + 1, dtype=np.int64) * NSLOT_CORE) << 20).astype(np.int64)
        ed = np.full((NC, 128 * EB), (200 << 20), np.int32)
        for k in range(NC):
            if not _nb_edge_scatter_core(packed[cuts[k]:cuts[k + 1]], ed[k],
                                         NT, EMAX, EB, k * NT):
                raise RuntimeError("tile overflow")
            put_piece(f"ed{side}", k, ed[k].reshape(128, EB))
        del packed
        return None
    ed = np.full(NC * 128 * EB, (200 << 20), np.int32)   # dummy: dloc=200, src=0
    if HAVE_NUMBA:
        if not _nb_edge_scatter(packed, ed, NT, EMAX, EB):
            raise RuntimeError("tile overflow")
        del packed
    else:
        srcs = (packed & 0xFFFFF).astype(np.int32)
        dss = (packed >> 20).astype(np.int32)
        del packed
        tile_g = dss >> 7                             # global tile id
        dloc = dss & 127
        NTILE_G = NC * NT
        tstart = np.searchsorted(dss, 128 * np.arange(NTILE_G + 1)).astype(np.int32)
        counts = np.diff(tstart)
        if counts.max() > EMAX * 128:
            raise RuntimeError(f"tile overflow: {counts.max()} > {EMAX * 128}")
        rank = np.arange(len(srcs), dtype=np.int32) - tstart[tile_g]
        core_e = tile_g // NT
        pos_e = ((core_e * 128 + (rank & 127)) * EB
                 + (tile_g - core_e * NT) * EMAX + (rank >> 7))
        ed[pos_e] = (dloc << 20) | srcs
    emit(f"ed{side}", ed.reshape(NC * 128, EB))

    r = dict(
        hp=hp_pad.reshape(NC, TSH, cfg["EMB"]),
        ed=ed.reshape(NC, 128, EB),
        bl=bl_sl.reshape(NC, 128, NT),
        invc=invc.reshape(NC, 128, WPC),
    )
    if hsc_big is not None:
        r["hsc"] = hsc_big.reshape(NC, 128, cfg["EMB"])
    return r


def _prepare_fast(nc, n_cores):
    """Build a cached jitted executable around the bass custom call so real
    calls can pass pre-sharded jax arrays (transfer overlapped with prep)."""
    import jax
    import concourse.mybir as mybir
    from concourse import bass2jax
    from jax.experimental.shard_map import shard_map
    from jax.sharding import Mesh, PartitionSpec, NamedSharding

    bass2jax.install_neuronx_cc_hook()
    assert nc.dbg_addr is None
    pid_name = (nc.partition_id_tensor.name
                if nc.partition_id_tensor is not None else None)

    in_names, out_names, out_avals, zero_shapes = [], [], [], []
    for alloc in nc.m.functions[0].allocations:
        if not isinstance(alloc, mybir.MemoryLocationSet):
            continue
        name = alloc.memorylocations[0].name
        if alloc.kind == "ExternalInput":
            if name != pid_name:
                in_names.append(name)
        elif alloc.kind == "ExternalOutput":
            out_names.append(name)
            shape = tuple(alloc.tensor_shape)
            dtype = mybir.dt.np(alloc.dtype)
            out_avals.append(jax.core.ShapedArray(shape, dtype))
            zero_shapes.append((shape, dtype))
    n_params = len(in_names)
    all_names = in_names + out_names
    if pid_name is not None:
        all_names = all_names + [pid_name]
    donate = tuple(range(n_params, n_params + len(out_names)))

    def _body(*args):
        operands = list(args)
        if pid_name is not None:
            operands.append(bass2jax.partition_id_tensor())
        outs = bass2jax._bass_exec_p.bind(
            *operands,
            out_avals=tuple(out_avals),
            in_names=tuple(all_names),
            out_names=tuple(out_names),
            lowering_input_output_aliases=(),
            sim_require_finite=True,
            sim_require_nnan=True,
            nc=nc,
        )
        return tuple(outs)

    devices = jax.devices()[:n_cores]
    mesh = Mesh(np.asarray(devices), ("core",))
    nspecs = n_params + len(out_names)
    sharded = jax.jit(
        shard_map(_body, mesh=mesh,
                  in_specs=(PartitionSpec("core"),) * nspecs,
                  out_specs=(PartitionSpec("core"),) * len(out_names),
                  check_rep=False),
        donate_argnums=donate, keep_unused=True)
    sh = NamedSharding(mesh, PartitionSpec("core"))
    return dict(sharded=sharded, in_names=in_names, out_names=out_names,
                zero_shapes=zero_shapes, sh=sh, n_cores=n_cores,
                device_put=jax.device_put, devices=devices,
                make_array=jax.make_array_from_single_device_arrays)


def _fast_exec(fast, arrays):
    """arrays: dict name -> jax array (pre-sharded) or np array (global
    concat layout). Returns list of np outputs in out_names order."""
    put = fast["device_put"]
    ops = []
    for name in fast["in_names"]:
        a = arrays[name]
        if isinstance(a, np.ndarray):
            a = put(a, fast["sh"])
        ops.append(a)
    zeros = [put(np.zeros((fast["n_cores"] * s[0], *s[1:]), d), fast["sh"])
             for (s, d) in fast["zero_shapes"]]
    outs = fast["sharded"](*ops, *zeros)
    return [np.asarray(o) for o in outs]


def run(cfg, nc, x_s, edge_index_s, x_s_batch, x_t, edge_index_t, x_t_batch, y,
        W_gcn, b_gcn, W_out, b_out):
    from concourse import bass_utils

    NC = cfg["N_CORES"]
    ps = _side_prep(cfg, x_s, edge_index_s, x_s_batch, W_gcn)
    pt = _side_prep(cfg, x_t, edge_index_t, x_t_batch, W_gcn)

    iota = np.broadcast_to(np.arange(128, dtype=np.float32).astype(BF16),
                           (128, 128)).copy()
    ident = np.eye(128, dtype=np.float32)
    bgr = np.broadcast_to(np.tile(np.asarray(b_gcn, np.float32), cfg["SUP"]),
                          (128, cfg["SUP"] * cfg["EMB"])).copy()
    wo = np.asarray(W_out, np.float32)
    bo = np.broadcast_to(np.asarray(b_out, np.float32),
                         (128, len(np.asarray(b_out)))).copy()

    in_maps = []
    for k in range(NC):
        m = {
            "iota": iota, "ident": ident, "bgr": bgr, "wo": wo, "bo": bo,
        }
        for s, p in (("s", ps), ("t", pt)):
            m[f"hp{s}"] = p["hp"][k]
            m[f"ed{s}"] = p["ed"][k]
            m[f"bl{s}"] = p["bl"][k]
            if "hsc" in p:
                m[f"hsc{s}"] = p["hsc"][k]
            m[f"invc{s}"] = p["invc"][k]
        in_maps.append(m)

    res = bass_utils.run_bass_kernel_spmd(nc, in_maps, core_ids=list(range(NC)))
    return np.concatenate([res.results[k]["out"] for k in range(NC)],
                          axis=0).astype(np.float32)


def _const_arrays(cfg, b_gcn, W_out, b_out):
    NC = cfg["N_CORES"]
    iota = np.broadcast_to(np.arange(128, dtype=np.float32).astype(BF16),
                           (128, 128)).copy()
    ident = np.eye(128, dtype=np.float32)
    bgr = np.broadcast_to(np.tile(np.asarray(b_gcn, np.float32), cfg["SUP"]),
                          (128, cfg["SUP"] * cfg["EMB"])).copy()
    wo = np.asarray(W_out, np.float32)
    bo = np.broadcast_to(np.asarray(b_out, np.float32),
                         (128, len(np.asarray(b_out)))).copy()
    return {
        "iota": np.tile(iota, (NC, 1)),
        "ident": np.tile(ident, (NC, 1)),
        "bgr": np.tile(bgr, (NC, 1)),
        "wo": np.tile(wo, (NC, 1)),
        "bo": np.tile(bo, (NC, 1)),
    }


def run_fast(cfg, fast, x_s, edge_index_s, x_s_batch, x_t, edge_index_t,
             x_t_batch, y, W_gcn, b_gcn, W_out, b_out):
    import os
    import time
    prof = bool(os.environ.get("KERNEL_PROF"))
    t0 = time.perf_counter()
    arrays = {}
    pieces = {}
    dput = fast["device_put"]
    devices = fast["devices"]

    def put(name, arr):
        arrays[name] = dput(arr, fast["sh"])

    def put_piece(name, k, piece):
        pieces.setdefault(name, {})[k] = dput(piece, devices[k])

    # donated output zero-buffers don't depend on inputs: ship them first
    zeros = [dput(np.zeros((fast["n_cores"] * s[0], *s[1:]), d), fast["sh"])
             for (s, d) in fast["zero_shapes"]]
    for name, arr in _const_arrays(cfg, b_gcn, W_out, b_out).items():
        put(name, arr)
    _side_prep(cfg, x_s, edge_index_s, x_s_batch, W_gcn, put=put,
               put_piece=put_piece, side="s")
    if prof:
        print(f"  prep s done: {time.perf_counter()-t0:.2f}s", flush=True)
    _side_prep(cfg, x_t, edge_index_t, x_t_batch, W_gcn, put=put,
               put_piece=put_piece, side="t")
    if prof:
        print(f"  prep t done: {time.perf_counter()-t0:.2f}s", flush=True)
    try:
        for name, pc in pieces.items():
            shards = [pc[k] for k in range(fast["n_cores"])]
            gshape = (sum(s.shape[0] for s in shards),) + shards[0].shape[1:]
            arrays[name] = fast["make_array"](gshape, fast["sh"], shards)
        ops = [arrays[n] for n in fast["in_names"]]
        outs = fast["sharded"](*ops, *zeros)
        outs = [np.asarray(o).astype(np.float32) for o in outs]
        if prof:
            print(f"  exec done:   {time.perf_counter()-t0:.2f}s", flush=True)
    finally:
        for a in arrays.values():
            try:
                a.delete()
            except Exception:
                pass
    return outs[0]


def kernel(x_s, edge_index_s, x_s_batch, x_t, edge_index_t, x_t_batch, y,
           W_gcn, b_gcn, W_out, b_out):
    cfg = _CACHE.get("cfg")
    if cfg is None:
        cfg = _derive(FULL_CFG)
        _CACHE["cfg"] = cfg
    nc = _CACHE.get("nc")
    if nc is None:
        nc = build_program(cfg)
        _CACHE["nc"] = nc
    fast = _CACHE.get("fast")
    if fast is not None:
        try:
            return run_fast(cfg, fast, x_s, edge_index_s, x_s_batch, x_t,
                            edge_index_t, x_t_batch, y, W_gcn, b_gcn, W_out,
                            b_out)
        except Exception:
            pass
    return run(cfg, nc, x_s, edge_index_s, x_s_batch, x_t, edge_index_t,
               x_t_batch, y, W_gcn, b_gcn, W_out, b_out)


def _warmup():
    """Compile the PJRT executable (and warm transfer paths) with dummy
    zero inputs so the first real kernel() call doesn't pay for it."""
    cfg = _CACHE["cfg"]
    fast = _CACHE["fast"]
    NC = cfg["N_CORES"]
    NT, WPC, EB, TSH = cfg["NT"], cfg["WPC"], cfg["EB"], cfg["TSH"]
    m = {
        "iota": np.zeros((NC * 128, 128), BF16),
        "ident": np.zeros((NC * 128, 128), np.float32),
        "bgr": np.zeros((NC * 128, cfg["SUP"] * cfg["EMB"]), np.float32),
        "wo": np.zeros((NC * 2 * cfg["EMB"], cfg["NCLS"]), np.float32),
        "bo": np.zeros((NC * 128, cfg["NCLS"]), np.float32),
    }
    if cfg.get("HP_INT8"):
        hp_np = np.int8
    elif cfg.get("HP_FP8"):
        hp_np = ml_dtypes.float8_e4m3
    else:
        hp_np = BF16
    for s in ("s", "t"):
        m[f"hp{s}"] = np.zeros((NC * TSH, cfg["EMB"]), hp_np)
        if cfg.get("HP_INT8"):
            m[f"hsc{s}"] = np.zeros((NC * 128, cfg["EMB"]), np.float32)
        m[f"ed{s}"] = np.zeros((NC * 128, EB), np.int32)
        m[f"bl{s}"] = np.zeros((NC * 128, NT), np.uint8)
        m[f"invc{s}"] = np.zeros((NC * 128, WPC), np.float32)
    _fast_exec(fast, m)


# compile at import so the first kernel() call doesn't pay for it
try:
    import os as _os
    if HAVE_NUMBA:
        _s = np.zeros(4, np.int32)
        _sl = np.zeros(4, np.int32)
        _p, _ = _nb_pack_deg(_s, _s, _sl, 4)
        _p.sort()
        _nb_edge_scatter(_p, np.zeros(128 * 19, np.int32), 1, 19, 19)
        _nb_edge_scatter_core(_p, np.zeros(128 * 19, np.int32), 1, 19, 19, 0)
        _nb_scale_bf16(np.zeros((2, 2), np.float32), np.ones(2, np.float32),
                       np.zeros((2, 2), np.uint16))
        _nb_scale_fp8(np.zeros((2, 2), np.float32), np.ones(2, np.float32),
                      np.zeros((2, 2), np.uint8))
        _nb_quant_i8(np.ones((2, 2), np.float32), np.ones(2, np.float32),
                     np.zeros((2, 2), np.int8), np.zeros(2, np.float32))
    _CACHE["cfg"] = _derive(FULL_CFG)
    _CACHE["nc"] = build_program(_CACHE["cfg"])
    try:
        _CACHE["fast"] = _prepare_fast(_CACHE["nc"], _CACHE["cfg"]["N_CORES"])
        if not _os.environ.get("KERNEL_NO_WARMUP"):
            _warmup()
    except Exception:
        _CACHE.pop("fast", None)
except Exception:
    _CACHE.pop("nc", None)
